# revision 35
# baseline (speedup 1.0000x reference)
"""Trainium2 Bass kernel for the masked multi-head attention module.

Shapes (hardcoded): B=4, SQ=SK=1024, D=1024, H=16, DH=64.
Sharding over 8 cores: core c -> batch b=c//2, head-half hh=c%2 (8 heads),
output-column-half hh. Pairwise AllGather of normalized ctx^T between cores
(2b, 2b+1), then each core computes a disjoint 512-column slice of the output.

v2 (vs baseline):
- bf16 operands everywhere (tolerance is 2e-2); halves DMA + DVE + SBUF.
- v_mask folded into V rows (incl. the softmax-denominator ones column)
  instead of an exp bias, so exp is a bare activation.
- Softmax normalization broadcast done with a tiny K=2 PE matmul
  (ones-selector lhsT) instead of DRAM round trips; collectives trigger
  within ~3us of each pair's ctx, overlapping the next pair's compute.
- Scores matmuls use K=64 row-split per head (no zero padding), letting the
  two heads of a pair run on disjoint PE row groups.
- Output projection split in two phases so only the last pair's 2 head-tiles
  depend on the final AllGather.
"""

import os
import numpy as np

B, S, D, H, DH = 4, 1024, 1024, 16, 64
P = 128

_CACHE = {}
LAST_RESULT = None


def _build_program():
    from concourse import bacc
    import concourse.bass as bass
    import concourse.tile as tile
    from concourse import mybir

    f32 = mybir.dt.float32
    f32r = mybir.dt.float32r
    bf16 = mybir.dt.bfloat16
    Exp = mybir.ActivationFunctionType.Exp

    nc = bacc.Bacc("TRN2", target_bir_lowering=False, debug=False, num_devices=8)

    # inputs are host-packed so every DMA partition line is 4 KB contiguous
    # (DMA here is packet-rate bound, not bandwidth bound)
    qT_in = nc.dram_tensor("qT_in", [4, P, 2 * S], bf16, kind="ExternalInput")
    vT_in = nc.dram_tensor("vT_in", [4, P, 2 * S], bf16, kind="ExternalInput")
    wq_d = nc.dram_tensor("wq", [2, P, 4, 512], bf16, kind="ExternalInput")
    wk_d = nc.dram_tensor("wk", [2, P, 4, 512], bf16, kind="ExternalInput")
    wv_d = nc.dram_tensor("wv", [2, P, 4, 512], bf16, kind="ExternalInput")
    wo_d = nc.dram_tensor("wo", [2, P, 4, 512], bf16, kind="ExternalInput")
    bq_d = nc.dram_tensor("bq2", [P, 4], f32, kind="ExternalInput")
    bk_d = nc.dram_tensor("bk2", [P, 4], f32, kind="ExternalInput")
    bv_d = nc.dram_tensor("bv_row", [1, 512], f32, kind="ExternalInput")
    bo_d = nc.dram_tensor("bo_row", [1, 512], f32, kind="ExternalInput")
    vm_d = nc.dram_tensor("vm8", [P, 8], f32, kind="ExternalInput")
    qm_d = nc.dram_tensor("qm8", [P, 8], f32, kind="ExternalInput")
    y_out = nc.dram_tensor("y_out", [S, 512], f32, kind="ExternalOutput")

    groups = [[0, 1], [2, 3], [4, 5], [6, 7]]

    def bcast_ap(src_ap, nparts):
        # partition-broadcast read (stride-0 partition dim); DRAM source only
        return bass.AP(
            tensor=src_ap.tensor,
            offset=src_ap.offset,
            ap=[[0, nparts]] + list(src_ap.ap[1:]),
        )

    with tile.TileContext(nc) as tc:
        with (
            tc.tile_pool(name="XB", bufs=8) as XB,        # qT/vT bf16 [128,2048]
            tc.tile_pool(name="HB", bufs=12) as HB,       # QTp/KT bf16 [128,1024]
            tc.tile_pool(name="CT", bufs=8) as CT,        # ctxT_full bf16 [128,1024]
            tc.tile_pool(name="W", bufs=8) as Wp,         # weights bf16 [128,4,512]
            tc.tile_pool(name="VS", bufs=8) as VSp,       # Vst bf16 [128,8,65]
            tc.tile_pool(name="UT", bufs=18) as UTp,      # exp outputs bf16 [128,1024]
            tc.tile_pool(name="ST", bufs=4) as STp,       # stU/st bf16 [128,1024]
            tc.tile_pool(name="YB", bufs=8) as YBp,       # out-proj partials bf16
            tc.tile_pool(name="YO", bufs=3) as YOp,       # final y fp32 [128,512]
            tc.tile_pool(name="SM", bufs=1) as SM,        # small consts
            tc.tile_pool(name="SM2", bufs=2) as SM2,      # sums/recip staging
            tc.tile_pool(name="ps_sc", bufs=2, space="PSUM") as PSC,   # 4 banks
            tc.tile_pool(name="ps_cx", bufs=2, space="PSUM") as PSX,   # 2 banks
            tc.tile_pool(name="ps_wk", bufs=2, space="PSUM") as PSW,   # 2 banks
            tc.tile_pool(name="dram", bufs=8, space="DRAM") as DR,
        ):
            # ---- small constants ----
            bq_sb = SM.tile([P, 4], f32, tag="bq")
            nc.gpsimd.dma_start(out=bq_sb[:], in_=bq_d[:, :])
            bk_sb = SM.tile([P, 4], f32, tag="bk")
            nc.gpsimd.dma_start(out=bk_sb[:], in_=bk_d[:, :])
            vm_sb = SM.tile([P, 8], f32, tag="vm")
            nc.gpsimd.dma_start(out=vm_sb[:], in_=vm_d[:, :])
            qm_sb = SM.tile([P, 8], f32, tag="qm")
            nc.gpsimd.dma_start(out=qm_sb[:], in_=qm_d[:, :])
            bv_bc = SM.tile([P, 512], f32, tag="bvb")
            nc.gpsimd.dma_start(out=bv_bc[:], in_=bcast_ap(bv_d[:, :], P))
            bo_bc = SM.tile([P, 512], f32, tag="bob")
            nc.gpsimd.dma_start(out=bo_bc[:], in_=bcast_ap(bo_d[:, :], P))
            # selectors for the per-head sums broadcast matmuls:
            # selA -> out partitions 0:64 (head A), selB -> 64:128 (head B)
            selA = SM.tile([1, P], f32, tag="selA")
            nc.vector.memset(selA[0:1, 0:64], 1.0)
            nc.vector.memset(selA[0:1, 64:128], 0.0)
            selB = SM.tile([1, P], f32, tag="selB")
            nc.vector.memset(selB[0:1, 0:64], 0.0)
            nc.vector.memset(selB[0:1, 64:128], 1.0)

            # ---- big loads (q/v pre-transposed + bf16 on host) ----
            qT = [None] * 8
            vT = [None] * 8
            wq_sb = [None] * 8
            wk_sb = [None] * 8
            wv_sb = [None] * 8
            wo_sb = [None] * 8

            def load_w(eng, w_dram):
                # one [128, 4, 512] tile per 512 D-rows; dst[di] is a view
                dst = []
                for j in range(2):
                    t = Wp.tile([P, 4, 512], bf16, tag="w")
                    eng.dma_start(out=t[:], in_=w_dram[j])
                    dst.append(t)
                return [dst[di // 4][:, di % 4, :] for di in range(8)]

            def load_x(eng, x_dram):
                dst = []
                for j in range(4):
                    t = XB.tile([P, 2 * S], bf16, tag="x")
                    eng.dma_start(out=t[:], in_=x_dram[j])
                    dst.append(t)
                return [dst[di // 2][:, (di % 2) * S:(di % 2 + 1) * S]
                        for di in range(8)]

            # concurrent load queues (both hardware DGE): K path on sync
            # (gates pair 0), Q path on scalar, V/O weights on gpsimd
            wk_sb = load_w(nc.sync, wk_d)
            vT = load_x(nc.sync, vT_in)
            wq_sb = load_w(nc.scalar, wq_d)
            qT = load_x(nc.scalar, qT_in)
            wv_sb = load_w(nc.gpsimd, wv_d)
            wo_sb = load_w(nc.gpsimd, wo_d)

            QTp = [None] * 8   # per-head Q^T [128, S]; head 2p in rows 0:64 of
                               # even tiles, head 2p+1 in rows 64:128 of odd
            KT = [None] * 4    # stacked K^T head pairs [128, S]
            Vst = [None] * 8   # masked V with ones column [128, 8, 65]

            def q_group(ht):
                tA = HB.tile([P, S], bf16, tag="h")
                tB = HB.tile([P, S], bf16, tag="h")
                for c in range(2):
                    cs = slice(c * 512, (c + 1) * 512)
                    ps = PSW.tile([P, 512], f32, tag="work")
                    for di in range(8):
                        nc.tensor.matmul(
                            ps[:, :],
                            lhsT=wq_sb[di][:, ht * P:(ht + 1) * P],
                            rhs=qT[di][:, cs],
                            start=(di == 0),
                            stop=(di == 7),
                        )
                    nc.vector.tensor_scalar_add(
                        tA[0:64, cs], ps[0:64, :], bq_sb[0:64, ht:ht + 1]
                    )
                    nc.vector.tensor_scalar_add(
                        tB[64:128, cs], ps[64:128, :], bq_sb[64:128, ht:ht + 1]
                    )
                QTp[2 * ht], QTp[2 * ht + 1] = tA, tB

            def k_group(ht):
                t = HB.tile([P, S], bf16, tag="h")
                for c in range(2):
                    cs = slice(c * 512, (c + 1) * 512)
                    ps = PSW.tile([P, 512], f32, tag="work")
                    for di in range(8):
                        nc.tensor.matmul(
                            ps[:, :],
                            lhsT=wk_sb[di][:, ht * P:(ht + 1) * P],
                            rhs=vT[di][:, cs],
                            start=(di == 0),
                            stop=(di == 7),
                        )
                    nc.vector.tensor_scalar_add(
                        t[:, cs], ps[:, :], bk_sb[:, ht:ht + 1]
                    )
                KT[ht] = t

            def v_group(kt):
                ps = PSW.tile([P, 512], f32, tag="work")
                for di in range(8):
                    nc.tensor.matmul(
                        ps[:, :],
                        lhsT=vT[di][:, kt * P:(kt + 1) * P],
                        rhs=wv_sb[di][:, :],
                        start=(di == 0),
                        stop=(di == 7),
                    )
                t = VSp.tile([P, 8, 65], bf16, tag="vst")
                nc.vector.memset(t[:], 1.0)
                nc.vector.tensor_add(
                    t[:, :, 0:64],
                    ps[:, :].rearrange("p (h d) -> p h d", h=8),
                    bv_bc[:, :].rearrange("p (h d) -> p h d", h=8),
                )
                # fold the key mask into V rows AND the ones column: masked
                # keys then contribute nothing to ctx nor to the softmax sum
                nc.vector.tensor_scalar_mul(t[:, :, :], t[:, :, :], vm_sb[:, kt:kt + 1])
                Vst[kt] = t

            ctxT_full = [None] * 8

            def pair_scores(p, uts):
                # scores + exp only (no Vst dependency): lets pair 0's exp
                # chain start as soon as K/Q projections land
                for c in range(2):
                    cs = slice(c * 512, (c + 1) * 512)
                    for kt in range(8):
                        ks = slice(kt * P, (kt + 1) * P)
                        sps = PSC.tile([P, S], f32, tag="sc")
                        # K=64 row-split: head A on PE rows 0:63, head B on
                        # rows 64:127 (disjoint row groups -> can overlap)
                        nc.tensor.matmul(
                            sps[:, 0:512],
                            lhsT=KT[p][0:64, ks],
                            rhs=QTp[2 * p][0:64, cs],
                            start=True,
                            stop=True,
                        )
                        nc.tensor.matmul(
                            sps[:, 512:1024],
                            lhsT=KT[p][64:128, ks],
                            rhs=QTp[2 * p + 1][64:128, cs],
                            start=True,
                            stop=True,
                        )
                        ut = UTp.tile([P, S], bf16, tag="ut")
                        nc.scalar.activation(ut[:], sps[:], Exp)
                        uts[(c, kt)] = ut

            def pair_ctx_finish(p, uts):
                stU = STp.tile([P, S], bf16, tag="st")
                smA = SM2.tile([1, S], f32, tag="sumA")
                smB = SM2.tile([1, S], f32, tag="sumB")
                for c in range(2):
                    cs = slice(c * 512, (c + 1) * 512)
                    ctxA = PSX.tile([65, 512], f32, tag="ctx")
                    ctxB = PSX.tile([65, 512], f32, tag="ctx")
                    for kt in range(8):
                        ut = uts[(c, kt)]
                        nc.tensor.matmul(
                            ctxA[:, :],
                            lhsT=Vst[kt][:, 2 * p, :],
                            rhs=ut[:, 0:512],
                            start=(kt == 0),
                            stop=(kt == 7),
                        )
                        nc.tensor.matmul(
                            ctxB[:, :],
                            lhsT=Vst[kt][:, 2 * p + 1, :],
                            rhs=ut[:, 512:1024],
                            start=(kt == 0),
                            stop=(kt == 7),
                        )
                    # evict unnormalized ctx + softmax sums out of PSUM
                    nc.vector.tensor_copy(stU[0:64, cs], ctxA[0:64, :])
                    nc.vector.tensor_copy(smA[0:1, cs].bitcast(f32r), ctxA[64:65, :])
                    nc.vector.tensor_copy(stU[64:128, cs], ctxB[0:64, :])
                    nc.vector.tensor_copy(smB[0:1, cs].bitcast(f32r), ctxB[64:65, :])

                # normalization: broadcast the per-head sums to all 128
                # partitions with two accumulating K=1 PE matmuls (0/1
                # selectors as lhsT), then take the reciprocal WIDE (128
                # lanes) and scale. q_mask is applied in the output
                # projection where queries are the partition dim.
                st = STp.tile([P, S], bf16, tag="st")
                for c in range(2):
                    cs = slice(c * 512, (c + 1) * 512)
                    # allocated from PSX (not PSW) so the projection pool's
                    # ring never chains later pairs' projections behind this
                    # pair's normalization
                    bc = PSX.tile([P, 512], f32, tag="ctx")
                    nc.tensor.matmul(
                        bc[:, :],
                        lhsT=selA[:, :].bitcast(f32r),
                        rhs=smA[:, cs].bitcast(f32r),
                        start=True, stop=False,
                    )
                    nc.tensor.matmul(
                        bc[:, :],
                        lhsT=selB[:, :].bitcast(f32r),
                        rhs=smB[:, cs].bitcast(f32r),
                        start=False, stop=True,
                    )
                    rb = SM2.tile([P, 512], f32, tag="rb")
                    nc.vector.reciprocal_approx_fast(rb[:], bc[:, :])
                    nc.vector.tensor_mul(st[:, cs], stU[:, cs], rb[:])

                # pairwise exchange of normalized ctx^T (bf16: 256 KB out)
                cin = DR.tile([P, S], bf16, tag="ccin")
                nc.gpsimd.dma_start(out=cin[:], in_=st[:])
                cout = DR.tile([2, P, S], bf16, tag="ccout")
                nc.gpsimd.collective_compute(
                    "AllGather",
                    mybir.AluOpType.bypass,
                    replica_groups=groups,
                    ins=[cin[:].opt()],
                    outs=[cout[:].opt()],
                )
                ta = CT.tile([P, S], bf16, tag="ct")
                nc.sync.dma_start(out=ta[:], in_=cout[0, :, :])
                tb = CT.tile([P, S], bf16, tag="ct")
                nc.sync.dma_start(out=tb[:], in_=cout[1, :, :])
                ctxT_full[p] = ta
                ctxT_full[4 + p] = tb

            def pair_compute_finish(p):
                uts = {}
                pair_scores(p, uts)
                pair_ctx_finish(p, uts)

            # ---- emit (order = scheduler priority: pair 0's scores/exp
            # before the V projection so exp starts the moment K/Q land) ----
            # tiny dummy AllGather up front absorbs the ~11us CC stream
            # warmup while the input DMAs run
            dmy_in = DR.tile([1, 4], f32, tag="dmy")
            nc.gpsimd.dma_start(out=dmy_in[:], in_=bq_d[0:1, 0:4])
            dmy_out = DR.tile([2, 1, 4], f32, tag="dmyo")
            nc.gpsimd.collective_compute(
                "AllGather",
                mybir.AluOpType.bypass,
                replica_groups=groups,
                ins=[dmy_in[:].opt()],
                outs=[dmy_out[:].opt()],
            )

            q_group(0)
            k_group(0)
            uts0 = {}
            pair_scores(0, uts0)
            for kt in range(8):
                v_group(kt)
            pair_ctx_finish(0, uts0)
            q_group(1)
            k_group(1)
            pair_compute_finish(1)
            q_group(2)
            k_group(2)
            pair_compute_finish(2)
            q_group(3)
            k_group(3)
            pair_compute_finish(3)

            # ---- output projection, two-phase ----
            # phase 1: head-tiles from pairs 0..2 (available during pair 3's
            # exchange); phase 2: pair 3's two head-tiles + bias + store
            HT1 = [0, 4, 1, 5, 2, 6]
            ybuf = [None] * 8
            for qt in range(8):
                qs = slice(qt * P, (qt + 1) * P)
                yp = PSW.tile([P, 512], f32, tag="work")
                for i, ht in enumerate(HT1):
                    nc.tensor.matmul(
                        yp[:, :],
                        lhsT=ctxT_full[ht][:, qs],
                        rhs=wo_sb[ht][:, :],
                        start=(i == 0),
                        stop=(i == len(HT1) - 1),
                    )
                yb = YBp.tile([P, 512], bf16, tag="yb")
                nc.vector.scalar_tensor_tensor(
                    yb[:], yp[:, :], qm_sb[:, qt:qt + 1], bo_bc[:],
                    mybir.AluOpType.mult, mybir.AluOpType.add,
                )
                ybuf[qt] = yb
            for qt in range(8):
                qs = slice(qt * P, (qt + 1) * P)
                yp = PSW.tile([P, 512], f32, tag="work")
                for i, ht in enumerate([3, 7]):
                    nc.tensor.matmul(
                        yp[:, :],
                        lhsT=ctxT_full[ht][:, qs],
                        rhs=wo_sb[ht][:, :],
                        start=(i == 0),
                        stop=(i == 1),
                    )
                ysb = YOp.tile([P, 512], f32, tag="yo")
                nc.vector.scalar_tensor_tensor(
                    ysb[:], yp[:, :], qm_sb[:, qt:qt + 1], ybuf[qt][:],
                    mybir.AluOpType.mult, mybir.AluOpType.add,
                )
                nc.scalar.dma_start(out=y_out[qs, :], in_=ysb[:])

    nc.compile()
    return nc


def _get_program():
    if "nc" not in _CACHE:
        _CACHE["nc"] = _build_program()
    return _CACHE["nc"]


def kernel(q, v, q_mask, v_mask, Wq, bq, Wk, bk, Wv, bv, Wo, bo):
    global LAST_RESULT
    from concourse.bass_utils import run_bass_kernel_spmd
    import ml_dtypes

    bf = ml_dtypes.bfloat16
    q = np.asarray(q, dtype=np.float32)
    v = np.asarray(v, dtype=np.float32)
    q_mask = np.asarray(q_mask)
    v_mask = np.asarray(v_mask)
    Wq = np.asarray(Wq, dtype=np.float32)
    Wk = np.asarray(Wk, dtype=np.float32)
    Wv = np.asarray(Wv, dtype=np.float32)
    Wo = np.asarray(Wo, dtype=np.float32)
    bq = np.asarray(bq, dtype=np.float32)
    bk = np.asarray(bk, dtype=np.float32)
    bv = np.asarray(bv, dtype=np.float32)
    bo = np.asarray(bo, dtype=np.float32)

    nc = _get_program()

    in_maps = []
    for c in range(8):
        b, hh = c // 2, c % 2
        hsl = slice(512 * hh, 512 * (hh + 1))
        vm = v_mask[b].astype(np.float32)
        qm = q_mask[b].astype(np.float32)

        def pack_x(xT):
            # [1024, 1024] -> [4, 128, 2048]: 4 KB DMA lines
            return np.ascontiguousarray(
                xT.reshape(4, 2, P, S).transpose(0, 2, 1, 3).reshape(4, P, 2 * S)
            ).astype(bf)

        def pack_w(w):
            # [1024, 512] -> [2, 128, 4, 512]: 4 KB DMA lines
            return np.ascontiguousarray(
                w.reshape(2, 4, P, 512).transpose(0, 2, 1, 3)
            ).astype(bf)

        in_maps.append(
            {
                "qT_in": pack_x(q[b].T),
                "vT_in": pack_x(v[b].T),
                "wq": pack_w(Wq[:, hsl]),
                "wk": pack_w(Wk[:, hsl]),
                "wv": pack_w(Wv[:, hsl]),
                "wo": pack_w(Wo[:, hsl]),
                "bq2": np.ascontiguousarray(bq[hsl].reshape(4, P).T),
                "bk2": np.ascontiguousarray(bk[hsl].reshape(4, P).T),
                "bv_row": np.ascontiguousarray(bv[hsl].reshape(1, 512)),
                "bo_row": np.ascontiguousarray(bo[hsl].reshape(1, 512)),
                "vm8": np.ascontiguousarray(vm.reshape(8, P).T),
                "qm8": np.ascontiguousarray(qm.reshape(8, P).T),
            }
        )

    td = os.environ.get("KERNEL_TRACE_DIR") or None
    if td:
        import tempfile

        td = tempfile.mkdtemp(dir=td)
    res = run_bass_kernel_spmd(
        nc,
        in_maps,
        core_ids=list(range(8)),
        tmpdir=td,
    )
    LAST_RESULT = res

    out = np.empty((B, S, D), dtype=np.float32)
    for b in range(B):
        out[b, :, 0:512] = res.results[2 * b]["y_out"]
        out[b, :, 512:1024] = res.results[2 * b + 1]["y_out"]
    return out
